# revision 9
# baseline (speedup 1.0000x reference)
"""Multi-head attention (RoPE, non-causal) Trainium2 Bass/Tile kernel.

Tensor-parallel over heads: each of 8 cores computes 2 of 16 heads plus its
partial out-projection; the host sums partial outputs (all-reduce).

Everything on-chip is oriented so no transposes are ever needed:
  - x arrives pre-transposed from host as xT [C, B*N]
  - Q,K are produced as QT/KT [hd, n] (head dim on partitions)
  - scores are computed transposed: sT[k,q] = K @ QT  (softmax over partitions,
    sums via ones-matmul on PE)
  - V is produced in natural [n, hd] layout -> AV gives OutT [hd, q]
  - OutT is exactly the lhsT the out-projection needs.

K production runs on the PE in fp8-e4m3 DoubleRow mode (2x MAC rate): the
host supplies x and W_k in a [128, ks, 2, n] pair layout packing 256
contraction dims per matmul; the psum->sbuf copy divides out the fp8
scaling. Q and V stay bf16 -- fp8 there would exceed the error budget
(fp8 K only perturbs softmax logits, where the error is ~1.4% rms).

The emission is software-pipelined for the in-order PE queue: the QKV
projection of batch i+1 is interleaved into the attention kb-steps of
batch i so the PE never waits on the ScalarE exp drain.
"""
import numpy as np
import ml_dtypes

import concourse.bacc as bacc
import concourse.bass as bass
import concourse.mybir as mybir
import concourse.tile as tile
from concourse.bass_utils import run_bass_kernel_spmd

P = 128
B, SEQ, C = 2, 2048, 2048
NB = B * SEQ            # 4096 total rows
HD = 128                # head dim
NHL = 2                 # heads per core
DL = NHL * HD           # 256 local head dims
KT = C // P             # 16 contraction tiles for QKV over C
KS = C // 256           # 8 pair-tiles for the fp8 DoubleRow K projection
QT = 512                # q tile (free dim of scores/AV)
NQT = SEQ // QT         # 4
KBN = SEQ // P          # 16 k blocks
ET = 512                # out-proj column tile
NET = C // ET           # 4
ROPE_BASE = 10000.0
SW, SX = 16.0, 32.0     # fp8 scale for W_k / x (undone in the psum copy)

BF = mybir.dt.bfloat16
F16 = mybir.dt.float16
F32 = mybir.dt.float32
E4 = mybir.dt.float8e4
AF = mybir.ActivationFunctionType
ALU = mybir.AluOpType
DRM = mybir.MatmulPerfMode.DoubleRow

EGROUP = 2             # exp group width in kb tiles (1, 2 or 4)
S_BUFS = 2             # score psum double-buffering


def build_kernel(reps=1):
    nc = bacc.Bacc("TRN2", target_bir_lowering=False, debug=False, num_devices=8)

    xt = nc.dram_tensor("xt", [C, NB], BF, kind="ExternalInput").ap()
    x8 = nc.dram_tensor("x8", [P, KS, 2, NB], E4, kind="ExternalInput").ap()
    wqv = nc.dram_tensor("wqv", [C, 4 * P], BF, kind="ExternalInput").ap()
    wk8 = nc.dram_tensor("wk8", [P, KS, 2, DL], E4, kind="ExternalInput").ap()
    wout = nc.dram_tensor("wout", [DL, C], BF, kind="ExternalInput").ap()
    tba = nc.dram_tensor("tba", [P, SEQ], BF, kind="ExternalInput").ap()
    tbb = nc.dram_tensor("tbb", [P, SEQ], BF, kind="ExternalInput").ap()
    out = nc.dram_tensor("out", [NB, C], F16, kind="ExternalOutput").ap()

    with tile.TileContext(nc) as tc:
        _emit(tc, xt, x8, wqv, wk8, wout, (tba, tbb), out, reps=reps)
    nc.compile()
    return nc


def _emit(tc, xt, x8, wqv, wk8, wout, tbls, out, reps=1):
    nc = tc.nc
    from contextlib import ExitStack
    ctx = ExitStack()
    with ctx:
        xt_pool = ctx.enter_context(tc.tile_pool(name="xt", bufs=2))
        x8_pool = ctx.enter_context(tc.tile_pool(name="x8", bufs=2))
        cst_pool = ctx.enter_context(tc.tile_pool(name="cst", bufs=1))
        qk_pool = ctx.enter_context(tc.tile_pool(name="qk", bufs=2))
        raw_pool = ctx.enter_context(tc.tile_pool(name="raw", bufs=3))
        tmp_pool = ctx.enter_context(tc.tile_pool(name="tmp", bufs=3))
        v_pool = ctx.enter_context(tc.tile_pool(name="v", bufs=2))
        pt_pool = ctx.enter_context(tc.tile_pool(name="pt", bufs=2))
        r_pool = ctx.enter_context(tc.tile_pool(name="r", bufs=2))
        rbc_pool = ctx.enter_context(tc.tile_pool(name="rbc", bufs=2))
        ot_pool = ctx.enter_context(tc.tile_pool(name="ot", bufs=2))
        stg_pool = ctx.enter_context(tc.tile_pool(name="stg", bufs=2))

        ps_misc = ctx.enter_context(tc.tile_pool(name="ps_misc", bufs=2, space="PSUM"))
        av_bufs = 8 - 2 - EGROUP * S_BUFS
        assert av_bufs >= 2
        ps_s = ctx.enter_context(tc.tile_pool(name="ps_s", bufs=S_BUFS, space="PSUM"))
        ps_av = ctx.enter_context(tc.tile_pool(name="ps_av", bufs=av_bufs, space="PSUM"))

        jobs = [(r, b) for r in range(reps) for b in range(B)]
        NJ = len(jobs)

        state = {}

        def new_xt_tile(job, nt, chunks=2):
            n0 = jobs[job][1] * SEQ
            t = xt_pool.tile([P, KT, QT], BF, tag="xt", name=f"xt_{job}_{nt}")
            src = xt[:, n0 + nt * QT: n0 + (nt + 1) * QT].rearrange(
                "(ko p) n -> p ko n", p=P)
            ck = KT // chunks
            for ci in range(chunks):
                nc.sync.dma_start(t[:, ci * ck:(ci + 1) * ck],
                                  src[:, ci * ck:(ci + 1) * ck])
            return t

        def new_x8_tile(job, nt):
            n0 = jobs[job][1] * SEQ
            t = x8_pool.tile([P, KS, 2, QT], E4, tag="x8", name=f"x8_{job}_{nt}")
            nc.sync.dma_start(t[:], x8[:, :, :, n0 + nt * QT: n0 + (nt + 1) * QT])
            return t

        tiles = {}

        def get_tiles(job, nt):
            if (job, nt) not in tiles:
                tiles[(job, nt)] = (new_xt_tile(job, nt), new_x8_tile(job, nt))
            return tiles.pop((job, nt))

        def prefetch_tiles(job, nt):
            """Issue the DMAs for (job, nt) ahead of its consumers."""
            nxt = (job, nt + 1) if nt + 1 < NQT else (job + 1, 0)
            if nxt[0] < NJ and nxt not in tiles:
                tiles[nxt] = (new_xt_tile(*nxt), new_x8_tile(*nxt))

        # --- startup: chunk the first xt tile and the first weight column
        # block so the first matmul group starts as early as possible ---
        wq_t = cst_pool.tile([P, KT, 4 * P], BF)
        wq_r = wqv.rearrange("(ko p) d -> p ko d", p=P)
        nc.sync.dma_start(wq_t[:, :, 0:P], wq_r[:, :, 0:P])
        xt0 = new_xt_tile(0, 0, chunks=4)
        nc.sync.dma_start(wq_t[:, :, P:2 * P], wq_r[:, :, P:2 * P])
        x80 = new_x8_tile(0, 0)
        wk8_t = cst_pool.tile([P, KS, 2, DL], E4)
        nc.sync.dma_start(wk8_t[:], wk8[:])
        nc.sync.dma_start(wq_t[:, :, 2 * P:4 * P], wq_r[:, :, 2 * P:4 * P])
        tb_a = cst_pool.tile([P, SEQ], BF)
        nc.sync.dma_start(tb_a[:], tbls[0][:])
        tb_b = cst_pool.tile([P, SEQ], BF)
        nc.sync.dma_start(tb_b[:], tbls[1][:])
        wo_t = cst_pool.tile([P, NHL, C], BF)
        nc.sync.dma_start(wo_t[:], wout.rearrange("(hl p) e -> p hl e", p=P))
        ones16 = cst_pool.tile([P, 1], F16)
        nc.vector.memset(ones16[:], 1.0)

        def alloc_job_tiles(job):
            state[job] = {
                "qrot": [qk_pool.tile([P, SEQ], BF, tag=f"qrot{h}",
                                      name=f"qrot{h}_{job}") for h in range(NHL)],
                "krot": [qk_pool.tile([P, SEQ], BF, tag=f"krot{h}",
                                      name=f"krot{h}_{job}") for h in range(NHL)],
                "v": v_pool.tile([P, KBN, DL], F16, tag="v", name=f"v_{job}"),
                "ot": ot_pool.tile([P, NHL, SEQ], BF, tag="ot", name=f"ot_{job}"),
            }

        def rope(dst, raw, nsl, label):
            H2 = HD // 2
            tmp = tmp_pool.tile([P, QT], BF, tag="tmp", name=f"tmp_{label}")
            # rot1 = x1*cos - x2*sin ; rot2 = x1*sin + x2*cos
            nc.vector.tensor_tensor(tmp[0:H2], raw[H2:P], tb_a[H2:P, nsl], ALU.mult)
            nc.vector.tensor_tensor(dst[0:H2, nsl], raw[0:H2], tb_a[0:H2, nsl], ALU.mult)
            nc.vector.tensor_tensor(dst[0:H2, nsl], dst[0:H2, nsl], tmp[0:H2], ALU.subtract)
            nc.vector.tensor_tensor(tmp[H2:P], raw[0:H2], tb_b[0:H2, nsl], ALU.mult)
            nc.vector.tensor_tensor(dst[H2:P, nsl], raw[H2:P], tb_b[H2:P, nsl], ALU.mult)
            nc.vector.tensor_tensor(dst[H2:P, nsl], dst[H2:P, nsl], tmp[H2:P], ALU.add)

        def emit_qkv_group(job, nt, gi):
            """One QKV matmul group. gi 0..1: Q heads (bf16); gi 2..3: K heads
            (fp8 DoubleRow); gi 4..7: V blocks (bf16)."""
            if gi == 0 and nt == 0:
                alloc_job_tiles(job)
            st = state[job]
            if gi == 0:
                st["xt"], st["x8"] = ((xt0, x80) if (job, nt) == (0, 0)
                                      else get_tiles(job, nt))
                prefetch_tiles(job, nt)
            xt_t = st["xt"]
            nsl = slice(nt * QT, (nt + 1) * QT)
            if gi < NHL:
                h = gi
                dst = st["qrot"][h]
                ps = ps_misc.tile([P, QT], F32, tag="ps", name=f"psq_{job}_{nt}_{gi}")
                for kc in range(KT):
                    nc.tensor.matmul(ps[:], wq_t[:, kc, h * P:(h + 1) * P],
                                     xt_t[:, kc],
                                     start=(kc == 0), stop=(kc == KT - 1))
                raw = raw_pool.tile([P, QT], BF, tag="raw", name=f"raw_{job}_{nt}_{gi}")
                nc.scalar.activation(raw[:], ps[:], AF.Copy)
                rope(dst, raw, nsl, f"q_{job}_{nt}_{gi}")
            elif gi < 2 * NHL:
                h = gi - NHL
                dst = st["krot"][h]
                x8_t = st["x8"]
                raw = raw_pool.tile([P, QT], BF, tag="raw", name=f"raw_{job}_{nt}_{gi}")
                for c2 in range(2):      # hd halves (DR out is 64 partitions)
                    ps = ps_misc.tile([64, 2, QT // 2], F32, tag="ps",
                                      name=f"psk_{job}_{nt}_{gi}_{c2}")
                    for th in range(2):
                        for ks in range(KS):
                            nc.tensor.matmul(
                                ps[:, th],
                                wk8_t[:, ks, :, h * HD + c2 * 64: h * HD + (c2 + 1) * 64],
                                x8_t[:, ks, :, th * (QT // 2):(th + 1) * (QT // 2)],
                                start=(ks == 0), stop=(ks == KS - 1),
                                perf_mode=DRM)
                    nc.scalar.activation(raw[c2 * 64:(c2 + 1) * 64, :],
                                         ps.rearrange("p a b -> p (a b)"),
                                         AF.Copy, scale=1.0 / (SW * SX))
                rope(dst, raw, nsl, f"k_{job}_{nt}_{gi}")
            else:
                nbl = gi - 2 * NHL
                ps = ps_misc.tile([P, DL], F32, tag="ps", name=f"psv_{job}_{nt}_{gi}")
                for kc in range(KT):
                    nc.tensor.matmul(ps[:], xt_t[:, kc, nbl * P:(nbl + 1) * P],
                                     wq_t[:, kc, 2 * P:4 * P],
                                     start=(kc == 0), stop=(kc == KT - 1))
                nc.scalar.activation(st["v"][:, nt * (QT // P) + nbl, :], ps[:], AF.Copy)

        deferred_proj = []
        pending_proj = []

        def emit_proj(job, qt, copy_tgl, nbls=None):
            st = state[job]
            n0 = jobs[job][1] * SEQ
            ot = st["ot"]
            for nbl in (nbls if nbls is not None else range(QT // P)):
                nb = qt * (QT // P) + nbl
                stg = stg_pool.tile([P, C], F16, tag="stg", name=f"stg_{job}_{nb}")
                for et in range(NET):
                    psp = ps_misc.tile([P, ET], F32, tag="ps",
                                       name=f"psp_{job}_{nb}_{et}")
                    for h in range(NHL):
                        nc.tensor.matmul(psp[:], ot[:, h, nb * P:(nb + 1) * P],
                                         wo_t[:, h, et * ET:(et + 1) * ET],
                                         start=(h == 0), stop=(h == NHL - 1))
                    nc.vector.tensor_copy(stg[:, et * ET:(et + 1) * ET], psp[:])
                nc.sync.dma_start(out[n0 + nb * P: n0 + (nb + 1) * P, :], stg[:])

        def emit_attention(job, filler):
            """Attention + out-proj for one batch; calls filler() between
            kb-steps to interleave next batch's QKV groups into PE order."""
            st = state[job]
            n0 = jobs[job][1] * SEQ
            qrot, krot, v_sb, ot = st["qrot"], st["krot"], st["v"], st["ot"]
            copy_tgl = [0]
            for qt in range(NQT):
                qsl = slice(qt * QT, (qt + 1) * QT)
                pt = [pt_pool.tile([P, KBN, QT], F16, tag="pt",
                                   name=f"pt{h}_{job}_{qt}") for h in range(NHL)]
                psav = [ps_av.tile([P, QT], F32, tag="av",
                                   name=f"psav{h}_{job}_{qt}") for h in range(NHL)]

                def scores_step(kb):
                    if kb % EGROUP:
                        return
                    for h in range(NHL):
                        if EGROUP == 1:
                            pss = ps_s.tile([P, QT], F32, tag="s",
                                            name=f"pss{h}_{job}_{qt}_{kb}")
                            nc.tensor.matmul(pss[:], krot[h][:, kb * P:(kb + 1) * P],
                                             qrot[h][:, qsl], start=True, stop=True)
                            nc.scalar.activation(pt[h][:, kb], pss[:], AF.Exp)
                        else:
                            pss = ps_s.tile([P, EGROUP, QT], F32, tag="s",
                                            name=f"pss{h}_{job}_{qt}_{kb}")
                            for j in range(EGROUP):
                                nc.tensor.matmul(
                                    pss[:, j], krot[h][:, (kb + j) * P:(kb + j + 1) * P],
                                    qrot[h][:, qsl], start=True, stop=True)
                            nc.scalar.activation(pt[h][:, kb:kb + EGROUP], pss[:], AF.Exp)

                def reduce_step(k0):
                    for h in range(NHL):
                        nc.tensor.matmul(psav[h][:], v_sb[:, k0, h * HD:(h + 1) * HD],
                                         pt[h][:, k0],
                                         start=(k0 == 0), stop=(k0 == KBN - 1))
                        # in-place pairwise sum tree over consumed pt slices;
                        # the final level runs on the idle Pool engine so the
                        # ones-matmul never waits behind the DVE queue
                        for lvl in (2, 4, 8, 16):
                            if (k0 + 1) % lvl == 0:
                                a, b = k0 + 1 - lvl, k0 + 1 - lvl // 2
                                eng = nc.gpsimd if lvl == 16 else nc.vector
                                eng.tensor_tensor(
                                    pt[h][:, a], pt[h][:, a], pt[h][:, b], ALU.add)

                off = EGROUP
                for kb in range(KBN + off):
                    if kb < KBN:
                        scores_step(kb)
                    if kb >= off:
                        reduce_step(kb - off)
                    if kb % 2 == 1 and kb >= 5:
                        filler()
                    if kb == 6 and pending_proj:
                        emit_proj(*pending_proj.pop(0), copy_tgl)
                filler()   # hide the sum-tree tail behind one QKV group
                pssum = []
                for h in range(NHL):
                    pss_h = ps_misc.tile([1, QT], F32, tag="ps",
                                         name=f"pssum{h}_{job}_{qt}")
                    nc.tensor.matmul(pss_h[:], ones16[:, :1],
                                     pt[h][:, 0], start=True, stop=True)
                    pssum.append(pss_h)
                # normalize -> OutT
                for h in range(NHL):
                    r = r_pool.tile([1, QT], F32, tag="r", name=f"r{h}_{job}_{qt}")
                    nc.vector.reciprocal(r[:], pssum[h][:])
                    rbc = rbc_pool.tile([P, QT], F32, tag="rbc", name=f"rbc{h}_{job}_{qt}")
                    nc.gpsimd.partition_broadcast(rbc[:], r[:])
                    nc.vector.tensor_tensor(ot[:, h, qsl], psav[h][:], rbc[:], ALU.mult)
                # out-projection: every q-tile is deferred to a later kb==6
                # hook (qt3 lands in the NEXT job's attention window) so proj
                # never waits on this qt's normalize chain
                pending_proj.append((job, qt))
                filler()

        # ---- schedule: QKV(0) up-front; attention(i) with QKV(i+1) fillers ----
        def make_filler(next_job):
            pending = []
            if next_job < NJ:
                pending = [(next_job, nt, gi) for nt in range(NQT)
                           for gi in range(2 * NHL + QT // P)]
            it = iter(pending)

            def filler():
                try:
                    emit_qkv_group(*next(it))
                except StopIteration:
                    pass
            return filler, it

        for nt in range(NQT):
            for gi in range(2 * NHL + QT // P):
                emit_qkv_group(0, nt, gi)
        for job in range(NJ):
            filler, it = make_filler(job + 1)
            emit_attention(job, filler)
            for rest in it:     # drain any leftover QKV groups
                emit_qkv_group(*rest)
        while pending_proj:     # drain the final job's deferred projections
            emit_proj(*pending_proj.pop(0), [0])
        while deferred_proj:    # safety: drain any leftover proj tiles
            emit_proj(*deferred_proj.pop(0), [0])


# ---------------------------------------------------------------------------
# host side
# ---------------------------------------------------------------------------

def host_tables():
    # sqrt(scale) folded into BOTH q and k rope tables
    s2 = (HD ** -0.5) ** 0.5
    inv_freq = 1.0 / (ROPE_BASE ** (np.arange(0, HD, 2, dtype=np.float64) / HD))
    t = np.arange(SEQ, dtype=np.float64)
    freqs = np.outer(inv_freq, t)          # [64, SEQ]
    cos = (np.cos(freqs) * s2).astype(np.float32)
    sin = (np.sin(freqs) * s2).astype(np.float32)
    tba = np.concatenate([cos, sin], 0)
    tbb = np.concatenate([sin, cos], 0)
    bf = ml_dtypes.bfloat16
    return tba.astype(bf), tbb.astype(bf)


def host_inputs(x, W_qkv, W_out):
    """Build per-core in_maps from full inputs."""
    bf = ml_dtypes.bfloat16
    e4 = ml_dtypes.float8_e4m3fn
    x2 = np.asarray(x, np.float32).reshape(NB, C)
    xt = np.ascontiguousarray(x2.T).astype(bf)
    # fp8 pair layout: contraction dim c = ks*256 + s*128 + p
    x8 = np.ascontiguousarray(
        (x2 * SX).reshape(NB, KS, 2, P).transpose(3, 1, 2, 0)).astype(e4)
    W_qkv = np.asarray(W_qkv, np.float32)
    W_out = np.asarray(W_out, np.float32)
    tba, tbb = host_tables()
    in_maps = []
    for c in range(8):
        h0, h1 = 2 * c, 2 * c + 1
        cols = []
        for three in (0, 2):        # Q and V head columns (bf16 path)
            for h in (h0, h1):
                cols.append(W_qkv[:, three * C + h * HD: three * C + (h + 1) * HD])
        wqv_c = np.concatenate(cols, 1).astype(bf)       # [C, 512]
        wk = np.concatenate(
            [W_qkv[:, C + h * HD: C + (h + 1) * HD] for h in (h0, h1)], 1)
        wk8 = np.ascontiguousarray(
            (wk * SW).reshape(KS, 2, P, DL).transpose(2, 0, 1, 3)).astype(e4)
        wo = W_out[h0 * HD:(h1 + 1) * HD, :].astype(bf)  # [256, C]
        in_maps.append({
            "xt": xt, "x8": x8, "wqv": wqv_c, "wk8": wk8, "wout": wo,
            "tba": tba, "tbb": tbb,
        })
    return in_maps


_NC_CACHE = {}


def get_nc(reps=1):
    if reps not in _NC_CACHE:
        _NC_CACHE[reps] = build_kernel(reps)
    return _NC_CACHE[reps]


def kernel_with_results(x, W_qkv, W_out, trace=False, **kw):
    nc = get_nc()
    in_maps = host_inputs(x, W_qkv, W_out)
    res = run_bass_kernel_spmd(nc, in_maps, core_ids=list(range(8)),
                               trace=trace, **kw)
    acc = np.zeros((NB, C), np.float64)
    for r in res.results:
        acc += r["out"].astype(np.float64)
    return acc.astype(np.float32).reshape(B, SEQ, C), res


def kernel(x, W_qkv, W_out):
    """Full-input / full-output MHA forward on 8 NeuronCores."""
    out, _ = kernel_with_results(x, W_qkv, W_out)
    return out


# revision 11
# speedup vs baseline: 8.3818x; 8.3818x over previous
"""Multi-head attention (RoPE, non-causal) Trainium2 Bass/Tile kernel.

Tensor-parallel over heads: each of 8 cores computes 2 of 16 heads plus its
partial out-projection; the host sums partial outputs (all-reduce).

Everything on-chip is oriented so no transposes are ever needed:
  - x arrives pre-transposed from host as xT [C, B*N]
  - Q,K are produced as QT/KT [hd, n] (head dim on partitions)
  - scores are computed transposed: sT[k,q] = K @ QT  (softmax over partitions,
    sums via ones-matmul on PE)
  - V is produced in natural [n, hd] layout -> AV gives OutT [hd, q]
  - OutT is exactly the lhsT the out-projection needs.

K production runs on the PE in fp8-e4m3 DoubleRow mode (2x MAC rate): the
host supplies x and W_k in a [128, ks, 2, n] pair layout packing 256
contraction dims per matmul; the psum->sbuf copy divides out the fp8
scaling. Q and V stay bf16 -- fp8 there would exceed the error budget
(fp8 K only perturbs softmax logits, where the error is ~1.4% rms).

The emission is software-pipelined for the in-order PE queue: the QKV
projection of batch i+1 is interleaved into the attention kb-steps of
batch i so the PE never waits on the ScalarE exp drain.
"""
import numpy as np
import ml_dtypes

import concourse.bacc as bacc
import concourse.bass as bass
import concourse.mybir as mybir
import concourse.tile as tile
from concourse.bass_utils import run_bass_kernel_spmd

P = 128
B, SEQ, C = 2, 2048, 2048
NB = B * SEQ            # 4096 total rows
HD = 128                # head dim
NHL = 2                 # heads per core
DL = NHL * HD           # 256 local head dims
KT = C // P             # 16 contraction tiles for QKV over C
KS = C // 256           # 8 pair-tiles for the fp8 DoubleRow K projection
QT = 512                # q tile (free dim of scores/AV)
NQT = SEQ // QT         # 4
KBN = SEQ // P          # 16 k blocks
ET = 512                # out-proj column tile
NET = C // ET           # 4
ROPE_BASE = 10000.0
SW, SX = 16.0, 32.0     # fp8 scale for W_k / x (undone in the psum copy)

BF = mybir.dt.bfloat16
F16 = mybir.dt.float16
F32 = mybir.dt.float32
E4 = mybir.dt.float8e4
AF = mybir.ActivationFunctionType
ALU = mybir.AluOpType
DRM = mybir.MatmulPerfMode.DoubleRow

EGROUP = 1             # exp group width in kb tiles (1, 2 or 4)
S_BUFS = 3             # score psum buffering


def build_kernel(reps=1):
    nc = bacc.Bacc("TRN2", target_bir_lowering=False, debug=False, num_devices=8)

    xt = nc.dram_tensor("xt", [C, NB], BF, kind="ExternalInput").ap()
    x8 = nc.dram_tensor("x8", [P, KS, 2, NB], E4, kind="ExternalInput").ap()
    wqv = nc.dram_tensor("wqv", [C, 4 * P], BF, kind="ExternalInput").ap()
    wk8 = nc.dram_tensor("wk8", [P, KS, 2, DL], E4, kind="ExternalInput").ap()
    wout = nc.dram_tensor("wout", [DL, C], BF, kind="ExternalInput").ap()
    tba = nc.dram_tensor("tba", [P, SEQ], BF, kind="ExternalInput").ap()
    tbb = nc.dram_tensor("tbb", [P, SEQ], BF, kind="ExternalInput").ap()
    out = nc.dram_tensor("out", [NB, C], F16, kind="ExternalOutput").ap()

    with tile.TileContext(nc) as tc:
        _emit(tc, xt, x8, wqv, wk8, wout, (tba, tbb), out, reps=reps)
    nc.compile()
    return nc


def _emit(tc, xt, x8, wqv, wk8, wout, tbls, out, reps=1):
    nc = tc.nc
    from contextlib import ExitStack
    ctx = ExitStack()
    with ctx:
        xt_pool = ctx.enter_context(tc.tile_pool(name="xt", bufs=2))
        x8_pool = ctx.enter_context(tc.tile_pool(name="x8", bufs=2))
        cst_pool = ctx.enter_context(tc.tile_pool(name="cst", bufs=1))
        qk_pool = ctx.enter_context(tc.tile_pool(name="qk", bufs=2))
        raw_pool = ctx.enter_context(tc.tile_pool(name="raw", bufs=3))
        tmp_pool = ctx.enter_context(tc.tile_pool(name="tmp", bufs=3))
        v_pool = ctx.enter_context(tc.tile_pool(name="v", bufs=2))
        pt_pool = ctx.enter_context(tc.tile_pool(name="pt", bufs=2))
        r_pool = ctx.enter_context(tc.tile_pool(name="r", bufs=2))
        rbc_pool = ctx.enter_context(tc.tile_pool(name="rbc", bufs=2))
        ot_pool = ctx.enter_context(tc.tile_pool(name="ot", bufs=2))
        stg_pool = ctx.enter_context(tc.tile_pool(name="stg", bufs=2))

        ps_misc = ctx.enter_context(tc.tile_pool(name="ps_misc", bufs=3, space="PSUM"))
        av_bufs = 8 - 3 - EGROUP * S_BUFS
        assert av_bufs >= 2
        ps_s = ctx.enter_context(tc.tile_pool(name="ps_s", bufs=S_BUFS, space="PSUM"))
        ps_av = ctx.enter_context(tc.tile_pool(name="ps_av", bufs=av_bufs, space="PSUM"))

        jobs = [(r, b) for r in range(reps) for b in range(B)]
        NJ = len(jobs)

        state = {}

        def new_xt_tile(job, nt, chunks=2):
            n0 = jobs[job][1] * SEQ
            t = xt_pool.tile([P, KT, QT], BF, tag="xt", name=f"xt_{job}_{nt}")
            src = xt[:, n0 + nt * QT: n0 + (nt + 1) * QT].rearrange(
                "(ko p) n -> p ko n", p=P)
            ck = KT // chunks
            for ci in range(chunks):
                nc.sync.dma_start(t[:, ci * ck:(ci + 1) * ck],
                                  src[:, ci * ck:(ci + 1) * ck])
            return t

        def new_x8_tile(job, nt):
            n0 = jobs[job][1] * SEQ
            t = x8_pool.tile([P, KS, 2, QT], E4, tag="x8", name=f"x8_{job}_{nt}")
            nc.sync.dma_start(t[:], x8[:, :, :, n0 + nt * QT: n0 + (nt + 1) * QT])
            return t

        tiles = {}

        def get_tiles(job, nt):
            if (job, nt) not in tiles:
                tiles[(job, nt)] = (new_xt_tile(job, nt), new_x8_tile(job, nt))
            return tiles.pop((job, nt))

        def prefetch_tiles(job, nt):
            """Issue the DMAs for (job, nt) ahead of its consumers."""
            nxt = (job, nt + 1) if nt + 1 < NQT else (job + 1, 0)
            if nxt[0] < NJ and nxt not in tiles:
                tiles[nxt] = (new_xt_tile(*nxt), new_x8_tile(*nxt))

        # --- startup: chunk the first xt tile and the first weight column
        # block so the first matmul group starts as early as possible ---
        wq_t = cst_pool.tile([P, KT, 4 * P], BF)
        wq_r = wqv.rearrange("(ko p) d -> p ko d", p=P)
        nc.sync.dma_start(wq_t[:, :, 0:P], wq_r[:, :, 0:P])
        xt0 = new_xt_tile(0, 0, chunks=4)
        nc.sync.dma_start(wq_t[:, :, P:2 * P], wq_r[:, :, P:2 * P])
        x80 = new_x8_tile(0, 0)
        wk8_t = cst_pool.tile([P, KS, 2, DL], E4)
        nc.sync.dma_start(wk8_t[:], wk8[:])
        nc.sync.dma_start(wq_t[:, :, 2 * P:4 * P], wq_r[:, :, 2 * P:4 * P])
        tb_a = cst_pool.tile([P, SEQ], BF)
        nc.sync.dma_start(tb_a[:], tbls[0][:])
        tb_b = cst_pool.tile([P, SEQ], BF)
        nc.sync.dma_start(tb_b[:], tbls[1][:])
        wo_t = cst_pool.tile([P, NHL, C], BF)
        nc.sync.dma_start(wo_t[:], wout.rearrange("(hl p) e -> p hl e", p=P))
        ones16 = cst_pool.tile([P, 1], F16)
        nc.vector.memset(ones16[:], 1.0)

        def alloc_job_tiles(job):
            state[job] = {
                "qrot": [qk_pool.tile([P, SEQ], BF, tag=f"qrot{h}",
                                      name=f"qrot{h}_{job}") for h in range(NHL)],
                "krot": [qk_pool.tile([P, SEQ], BF, tag=f"krot{h}",
                                      name=f"krot{h}_{job}") for h in range(NHL)],
                "v": v_pool.tile([P, KBN, DL], F16, tag="v", name=f"v_{job}"),
                "ot": ot_pool.tile([P, NHL, SEQ], BF, tag="ot", name=f"ot_{job}"),
            }

        def rope(dst, raw, nsl, label):
            H2 = HD // 2
            tmp = tmp_pool.tile([P, QT], BF, tag="tmp", name=f"tmp_{label}")
            # rot1 = x1*cos - x2*sin ; rot2 = x1*sin + x2*cos
            nc.vector.tensor_tensor(tmp[0:H2], raw[H2:P], tb_a[H2:P, nsl], ALU.mult)
            nc.vector.tensor_tensor(dst[0:H2, nsl], raw[0:H2], tb_a[0:H2, nsl], ALU.mult)
            nc.vector.tensor_tensor(dst[0:H2, nsl], dst[0:H2, nsl], tmp[0:H2], ALU.subtract)
            nc.vector.tensor_tensor(tmp[H2:P], raw[0:H2], tb_b[0:H2, nsl], ALU.mult)
            nc.vector.tensor_tensor(dst[H2:P, nsl], raw[H2:P], tb_b[H2:P, nsl], ALU.mult)
            nc.vector.tensor_tensor(dst[H2:P, nsl], dst[H2:P, nsl], tmp[H2:P], ALU.add)

        def emit_qkv_group(job, nt, gi):
            """One QKV matmul group. gi 0..1: Q heads (bf16); gi 2..3: K heads
            (fp8 DoubleRow); gi 4..7: V blocks (bf16)."""
            if gi == 0 and nt == 0:
                alloc_job_tiles(job)
            st = state[job]
            if gi == 0:
                st["xt"], st["x8"] = ((xt0, x80) if (job, nt) == (0, 0)
                                      else get_tiles(job, nt))
                prefetch_tiles(job, nt)
            xt_t = st["xt"]
            nsl = slice(nt * QT, (nt + 1) * QT)
            if gi < NHL:
                h = gi
                dst = st["qrot"][h]
                ps = ps_misc.tile([P, QT], F32, tag="ps", name=f"psq_{job}_{nt}_{gi}")
                for kc in range(KT):
                    nc.tensor.matmul(ps[:], wq_t[:, kc, h * P:(h + 1) * P],
                                     xt_t[:, kc],
                                     start=(kc == 0), stop=(kc == KT - 1))
                raw = raw_pool.tile([P, QT], BF, tag="raw", name=f"raw_{job}_{nt}_{gi}")
                nc.scalar.activation(raw[:], ps[:], AF.Copy)
                rope(dst, raw, nsl, f"q_{job}_{nt}_{gi}")
            elif gi < 2 * NHL:
                h = gi - NHL
                dst = st["krot"][h]
                x8_t = st["x8"]
                raw = raw_pool.tile([P, QT], BF, tag="raw", name=f"raw_{job}_{nt}_{gi}")
                for c2 in range(2):      # hd halves (DR out is 64 partitions)
                    ps = ps_misc.tile([64, 2, QT // 2], F32, tag="ps",
                                      name=f"psk_{job}_{nt}_{gi}_{c2}")
                    for th in range(2):
                        for ks in range(KS):
                            nc.tensor.matmul(
                                ps[:, th],
                                wk8_t[:, ks, :, h * HD + c2 * 64: h * HD + (c2 + 1) * 64],
                                x8_t[:, ks, :, th * (QT // 2):(th + 1) * (QT // 2)],
                                start=(ks == 0), stop=(ks == KS - 1),
                                perf_mode=DRM)
                    nc.scalar.activation(raw[c2 * 64:(c2 + 1) * 64, :],
                                         ps.rearrange("p a b -> p (a b)"),
                                         AF.Copy, scale=1.0 / (SW * SX))
                rope(dst, raw, nsl, f"k_{job}_{nt}_{gi}")
            else:
                nbl = gi - 2 * NHL
                ps = ps_misc.tile([P, DL], F32, tag="ps", name=f"psv_{job}_{nt}_{gi}")
                for kc in range(KT):
                    nc.tensor.matmul(ps[:], xt_t[:, kc, nbl * P:(nbl + 1) * P],
                                     wq_t[:, kc, 2 * P:4 * P],
                                     start=(kc == 0), stop=(kc == KT - 1))
                nc.scalar.activation(st["v"][:, nt * (QT // P) + nbl, :], ps[:], AF.Copy)

        deferred_proj = []
        pending_proj = []

        def emit_proj(job, qt, copy_tgl, nbls=None):
            st = state[job]
            n0 = jobs[job][1] * SEQ
            ot = st["ot"]
            for nbl in (nbls if nbls is not None else range(QT // P)):
                nb = qt * (QT // P) + nbl
                stg = stg_pool.tile([P, C], F16, tag="stg", name=f"stg_{job}_{nb}")
                for et in range(NET):
                    psp = ps_misc.tile([P, ET], F32, tag="ps",
                                       name=f"psp_{job}_{nb}_{et}")
                    for h in range(NHL):
                        nc.tensor.matmul(psp[:], ot[:, h, nb * P:(nb + 1) * P],
                                         wo_t[:, h, et * ET:(et + 1) * ET],
                                         start=(h == 0), stop=(h == NHL - 1))
                    nc.vector.tensor_copy(stg[:, et * ET:(et + 1) * ET], psp[:])
                nc.sync.dma_start(out[n0 + nb * P: n0 + (nb + 1) * P, :], stg[:])

        def emit_attention(job, filler):
            """Attention + out-proj for one batch; calls filler() between
            kb-steps to interleave next batch's QKV groups into PE order."""
            st = state[job]
            n0 = jobs[job][1] * SEQ
            qrot, krot, v_sb, ot = st["qrot"], st["krot"], st["v"], st["ot"]
            copy_tgl = [0]
            for qt in range(NQT):
                qsl = slice(qt * QT, (qt + 1) * QT)
                pt = [pt_pool.tile([P, KBN, QT], F16, tag="pt",
                                   name=f"pt{h}_{job}_{qt}") for h in range(NHL)]
                psav = [ps_av.tile([P, QT], F32, tag="av",
                                   name=f"psav{h}_{job}_{qt}") for h in range(NHL)]

                def scores_step(kb):
                    if kb % EGROUP:
                        return
                    for h in range(NHL):
                        if EGROUP == 1:
                            pss = ps_s.tile([P, QT], F32, tag="s",
                                            name=f"pss{h}_{job}_{qt}_{kb}")
                            nc.tensor.matmul(pss[:], krot[h][:, kb * P:(kb + 1) * P],
                                             qrot[h][:, qsl], start=True, stop=True)
                            nc.scalar.activation(pt[h][:, kb], pss[:], AF.Exp)
                        else:
                            pss = ps_s.tile([P, EGROUP, QT], F32, tag="s",
                                            name=f"pss{h}_{job}_{qt}_{kb}")
                            for j in range(EGROUP):
                                nc.tensor.matmul(
                                    pss[:, j], krot[h][:, (kb + j) * P:(kb + j + 1) * P],
                                    qrot[h][:, qsl], start=True, stop=True)
                            nc.scalar.activation(pt[h][:, kb:kb + EGROUP], pss[:], AF.Exp)

                def reduce_step(k0):
                    for h in range(NHL):
                        nc.tensor.matmul(psav[h][:], v_sb[:, k0, h * HD:(h + 1) * HD],
                                         pt[h][:, k0],
                                         start=(k0 == 0), stop=(k0 == KBN - 1))
                        # in-place pairwise sum tree over consumed pt slices;
                        # the final level runs on the idle Pool engine so the
                        # ones-matmul never waits behind the DVE queue
                        for lvl in (2, 4, 8, 16):
                            if (k0 + 1) % lvl == 0:
                                a, b = k0 + 1 - lvl, k0 + 1 - lvl // 2
                                eng = nc.gpsimd if lvl >= 4 else nc.vector
                                eng.tensor_tensor(
                                    pt[h][:, a], pt[h][:, a], pt[h][:, b], ALU.add)

                off = EGROUP
                for kb in range(KBN + off):
                    if kb < KBN:
                        scores_step(kb)
                    if kb >= off:
                        reduce_step(kb - off)
                    if kb % 2 == 1 and kb >= 5:
                        filler()
                    if kb == 6 and pending_proj:
                        emit_proj(*pending_proj.pop(0), copy_tgl)
                filler()   # hide the sum-tree tail behind one QKV group
                pssum = []
                for h in range(NHL):
                    pss_h = ps_misc.tile([1, QT], F32, tag="ps",
                                         name=f"pssum{h}_{job}_{qt}")
                    nc.tensor.matmul(pss_h[:], ones16[:, :1],
                                     pt[h][:, 0], start=True, stop=True)
                    pssum.append(pss_h)
                # normalize -> OutT
                for h in range(NHL):
                    r = r_pool.tile([1, QT], F32, tag="r", name=f"r{h}_{job}_{qt}")
                    nc.vector.reciprocal(r[:], pssum[h][:])
                    rbc = rbc_pool.tile([P, QT], F32, tag="rbc", name=f"rbc{h}_{job}_{qt}")
                    nc.gpsimd.partition_broadcast(rbc[:], r[:])
                    nc.vector.tensor_tensor(ot[:, h, qsl], psav[h][:], rbc[:], ALU.mult)
                # out-projection: every q-tile is deferred to a later kb==6
                # hook (qt3 lands in the NEXT job's attention window) so proj
                # never waits on this qt's normalize chain
                pending_proj.append((job, qt))
                filler()

        # ---- schedule: QKV(0) up-front; attention(i) with QKV(i+1) fillers ----
        def make_filler(next_job):
            pending = []
            if next_job < NJ:
                pending = [(next_job, nt, gi) for nt in range(NQT)
                           for gi in range(2 * NHL + QT // P)]
            it = iter(pending)

            def filler():
                try:
                    emit_qkv_group(*next(it))
                except StopIteration:
                    pass
            return filler, it

        for nt in range(NQT):
            for gi in range(2 * NHL + QT // P):
                emit_qkv_group(0, nt, gi)
        for job in range(NJ):
            filler, it = make_filler(job + 1)
            emit_attention(job, filler)
            for rest in it:     # drain any leftover QKV groups
                emit_qkv_group(*rest)
        while pending_proj:     # drain the final job's deferred projections
            emit_proj(*pending_proj.pop(0), [0])
        while deferred_proj:    # safety: drain any leftover proj tiles
            emit_proj(*deferred_proj.pop(0), [0])


# ---------------------------------------------------------------------------
# host side
# ---------------------------------------------------------------------------

def host_tables():
    # sqrt(scale) folded into BOTH q and k rope tables
    s2 = (HD ** -0.5) ** 0.5
    inv_freq = 1.0 / (ROPE_BASE ** (np.arange(0, HD, 2, dtype=np.float64) / HD))
    t = np.arange(SEQ, dtype=np.float64)
    freqs = np.outer(inv_freq, t)          # [64, SEQ]
    cos = (np.cos(freqs) * s2).astype(np.float32)
    sin = (np.sin(freqs) * s2).astype(np.float32)
    tba = np.concatenate([cos, sin], 0)
    tbb = np.concatenate([sin, cos], 0)
    bf = ml_dtypes.bfloat16
    return tba.astype(bf), tbb.astype(bf)


def host_inputs(x, W_qkv, W_out):
    """Build per-core in_maps from full inputs."""
    bf = ml_dtypes.bfloat16
    e4 = ml_dtypes.float8_e4m3fn
    x2 = np.asarray(x, np.float32).reshape(NB, C)
    xt = np.ascontiguousarray(x2.T).astype(bf)
    # fp8 pair layout: contraction dim c = ks*256 + s*128 + p
    x8 = np.ascontiguousarray(
        (x2 * SX).reshape(NB, KS, 2, P).transpose(3, 1, 2, 0)).astype(e4)
    W_qkv = np.asarray(W_qkv, np.float32)
    W_out = np.asarray(W_out, np.float32)
    tba, tbb = host_tables()
    in_maps = []
    for c in range(8):
        h0, h1 = 2 * c, 2 * c + 1
        cols = []
        for three in (0, 2):        # Q and V head columns (bf16 path)
            for h in (h0, h1):
                cols.append(W_qkv[:, three * C + h * HD: three * C + (h + 1) * HD])
        wqv_c = np.concatenate(cols, 1).astype(bf)       # [C, 512]
        wk = np.concatenate(
            [W_qkv[:, C + h * HD: C + (h + 1) * HD] for h in (h0, h1)], 1)
        wk8 = np.ascontiguousarray(
            (wk * SW).reshape(KS, 2, P, DL).transpose(2, 0, 1, 3)).astype(e4)
        wo = W_out[h0 * HD:(h1 + 1) * HD, :].astype(bf)  # [256, C]
        in_maps.append({
            "xt": xt, "x8": x8, "wqv": wqv_c, "wk8": wk8, "wout": wo,
            "tba": tba, "tbb": tbb,
        })
    return in_maps


_NC_CACHE = {}


def get_nc(reps=1):
    if reps not in _NC_CACHE:
        _NC_CACHE[reps] = build_kernel(reps)
    return _NC_CACHE[reps]


def kernel_with_results(x, W_qkv, W_out, trace=False, **kw):
    nc = get_nc()
    in_maps = host_inputs(x, W_qkv, W_out)
    res = run_bass_kernel_spmd(nc, in_maps, core_ids=list(range(8)),
                               trace=trace, **kw)
    acc = np.zeros((NB, C), np.float64)
    for r in res.results:
        acc += r["out"].astype(np.float64)
    return acc.astype(np.float32).reshape(B, SEQ, C), res


def kernel(x, W_qkv, W_out):
    """Full-input / full-output MHA forward on 8 NeuronCores."""
    out, _ = kernel_with_results(x, W_qkv, W_out)
    return out
